# revision 30
# baseline (speedup 1.0000x reference)
"""Trainium2 Bass kernel for the pointer-generator decoder step.

Strategy (8 NeuronCores):
  Phase A (batch-parallel, 8 rows/core): embedding gather, input projection,
  bidirectional-state reduction, LSTM cell, coverage attention, p_gen,
  pre-generation projection.  All activations are kept feature-major
  ("xT" = x.T) so PE matmuls chain without transposes.
  AllGather a small bundle (att_dist, pregen, p_gen) so every core sees the
  full batch.
  Phase B (vocab-parallel, 6275 cols/core): generation logits + exp, a tiny
  AllReduce for the softmax denominator, scale by p_gen, and an indirect-DMA
  scatter-add of the attention distribution into the extended vocabulary.
Host only shards/transposes inputs and concatenates per-core outputs.
"""

import sys

if "/opt/trn_rl_repo" not in sys.path:
    sys.path.insert(0, "/opt/trn_rl_repo")

import numpy as np

import concourse.bass as bass
import concourse.bacc as bacc
import concourse.mybir as mybir
from concourse import tile
from concourse.bass_utils import run_bass_kernel_spmd
from concourse.masks import make_identity
from concourse.tile_rust import add_dep_helper

F32 = mybir.dt.float32
I32 = mybir.dt.int32
AF = mybir.ActivationFunctionType

NCORES = 8
B, L, D, E, V, EX = 64, 400, 256, 128, 50000, 200
BL = B // NCORES            # 8 rows per core
VE = V + EX                 # 50200
VSL = VE // NCORES          # 6275 output columns per core
VPAD = 6400                 # padded slice width (12x512 + 1x256 chunks)
BLL = BL * L                # 3200
OOB = (1 << 30)             # scatter index sentinel (beyond bounds check)
NEG = -10000.0              # pad-column bias: exp underflows to exactly 0

KP_NW = [512] * 6 + [128]            # K-projection column chunks (sum = 3200)
NMCH = VPAD // 128                   # 50 vocab-column chunks of 128 (transposed)

_CACHE = {}
TRACE = False           # set by test harness to capture an NTFF profile
LAST = None             # BassKernelResults of the most recent run


# --------------------------------------------------------------------------
# program construction
# --------------------------------------------------------------------------

def _build(layer_tiles):
    """layer_tiles: tuple of per-layer scatter tile counts, e.g. (20, 5, 1, 1)."""
    nc = bacc.Bacc("TRN2", target_bir_lowering=False, debug=False,
                   num_devices=NCORES)

    def din(name, shape, dt=F32):
        return nc.dram_tensor(name, shape, dt, kind="ExternalInput")

    def dout(name, shape, dt=F32):
        return nc.dram_tensor(name, shape, dt, kind="ExternalOutput")

    ids = din("ids", [BL, 1], I32)
    small3 = din("small3", [12 * 128, BL])       # pctxT(4) hsT(4) csT(4)
    cov_aug = din("cov_aug", [2, BLL])           # [coverage; ones]
    enc_fm = din("enc_fm", [512, BLL])           # encoder slice, feature-major
    enc_rmp = din("enc_rmp", [BL * 512, 512])    # row-major, l padded to 512
    emb_w = din("emb_w", [V, E])
    wpack = din("wpack", [27 * 128, 256])        # lin5 redh4 redc4 wq4 wk4 pre6
    bias_pack = din("bias_pack", [5, 256])       # lin red_h red_c wq pre
    uv_wk = din("uv_wk", [2, 256])               # [wk@cov_w ; wk@cov_b+wk_b]
    gpack = din("gpack", [4 * 128, 1024])        # w_ih.T(2) w_hh.T(2)
    bias_g = din("bias_g", [1, 1024])            # b_ih + b_hh
    pgen_wT = din("pgen_wT", [10 * 128, 1])
    pgen_bb = din("pgen_bb", [BL, 2])            # col0 = +b, col1 = -b
    gen_aug = din("gen_aug", [257, VPAD])        # [gen_w_slice.T ; gen_b row]
    ntiles = sum(layer_tiles)
    s_pack = din("s_pack", [ntiles * 512, 128], mybir.dt.bfloat16)
    bsel = din("bsel", [ntiles * 128, 1])        # batch index per scatter row
    rowidx = din("rowidx", [ntiles * 128, 1], I32)
    iota64 = din("iota64", [128, B])

    finT = dout("finT", [VPAD, B])               # transposed final slice
    h_outT = dout("h_outT", [D, BL])
    c_outT = dout("c_outT", [D, BL])
    wc_out = dout("wc_out", [BL, 2 * D])
    att_out = dout("att_out", [BL, L])
    pgen_out = dout("pgen_out", [BL, 1])

    with tile.TileContext(nc) as tc:
        _emit(nc, tc, layer_tiles, locals())
    nc.compile()
    return nc


def _emit(nc, tc, layer_tiles, t):
    ctxs = []

    def pool(**kw):
        p = tc.tile_pool(**kw)
        ctxs.append(p)
        return p.__enter__()

    sb = pool(name="sb", bufs=1)              # resident tiles
    sbw = pool(name="sbw", bufs=2)            # working tiles
    ps = pool(name="ps", bufs=2, space="PSUM")        # all small psums: 2 banks
    ps_k = pool(name="ps_k", bufs=2, space="PSUM")    # K-proj: 2 banks
    ps_v = pool(name="ps_v", bufs=2, space="PSUM")    # gen chunks: 2 banks
    ps_sum = pool(name="ps_sum", bufs=1, space="PSUM")  # vocab sums: 1 bank
    dram = pool(name="dram", bufs=1, space="DRAM")
    # phase-A-only pools, released before phase-B pools are created
    pa_cm = tc.tile_pool(name="pa", bufs=1)
    pa = pa_cm.__enter__()
    est_cm = tc.tile_pool(name="est", bufs=2)
    est = est_cm.__enter__()

    ident = sb.tile([128, 128], F32)
    make_identity(nc, ident[:])
    ones8 = sb.tile([1, BL], F32)
    nc.vector.memset(ones8[:], 1.0)
    ones64 = sb.tile([1, B], F32)
    nc.vector.memset(ones64[:], 1.0)

    # ---------------- loads ----------------
    wp = pa.tile([128, 27, 256], F32)
    nc.sync.dma_start(out=wp[:], in_=t["wpack"][:].rearrange(
        "(k p) f -> p k f", p=128))
    S_LIN, S_RH, S_RC, S_WQ, S_WK, S_PRE = 0, 5, 9, 13, 17, 21

    gp = pa.tile([128, 4, 1024], F32)
    nc.sync.dma_start(out=gp[:], in_=t["gpack"][:].rearrange(
        "(k p) f -> p k f", p=128))
    bg = pa.tile([1, 1024], F32)
    nc.sync.dma_start(out=bg[:], in_=t["bias_g"][:])
    bp = pa.tile([1, 5, 256], F32)
    nc.sync.dma_start(out=bp[:], in_=t["bias_pack"][:].rearrange(
        "r f -> (r f)")[None, :])
    uvw = pa.tile([2, 256], F32)
    nc.sync.dma_start(out=uvw[:], in_=t["uv_wk"][:])
    pgw = pa.tile([128, 10], F32)
    nc.sync.dma_start(out=pgw[:], in_=t["pgen_wT"][:].rearrange(
        "(k p) o -> p (k o)", p=128))
    pgb = sb.tile([BL, 2], F32)
    nc.sync.dma_start(out=pgb[:], in_=t["pgen_bb"][:])

    s3 = sb.tile([128, 12, BL], F32)
    nc.sync.dma_start(out=s3[:], in_=t["small3"][:].rearrange(
        "(k p) b -> p k b", p=128))

    efm = pa.tile([128, 5, BLL], F32)
    nc.scalar.dma_start(out=efm[:, 0:4, :], in_=t["enc_fm"][:].rearrange(
        "(k p) n -> p k n", p=128))
    nc.sync.dma_start(out=efm[0:2, 4, :], in_=t["cov_aug"][:])

    ids_sb = sb.tile([BL, 1], I32)
    nc.sync.dma_start(out=ids_sb[:], in_=t["ids"][:])
    emb_sb = sb.tile([BL, E], F32)
    nc.gpsimd.indirect_dma_start(
        out=emb_sb[:], out_offset=None, in_=t["emb_w"][:],
        in_offset=bass.IndirectOffsetOnAxis(ap=ids_sb[:, :1], axis=0))

    # ---------------- phase A ----------------
    def mm_chain(out_ps, steps):
        last = len(steps) - 1
        for i, (lhsT, rhs) in enumerate(steps):
            nc.tensor.matmul(out_ps, lhsT, rhs, start=(i == 0), stop=(i == last))

    def tr(src_ap, p_out, n_out):
        """PE-transpose src [p_in, n_out] -> psum [n_out, p_in]."""
        pt = ps.tile([p_out, n_out], F32, tag="small")
        nc.tensor.transpose(pt[:], src_ap, ident[:src_ap.shape[0], :n_out])
        return pt

    # embT [128, 8]
    embT_ps = tr(emb_sb[:], E, BL)
    embT = sb.tile([E, BL], F32)
    nc.scalar.copy(out=embT[:], in_=embT_ps[:])

    def a_matmul(wsec, nk, rhs_list, bias_row, act, out_tag):
        """out[_m][128, BL] = act(sum_k wpack[wsec+k][:,mslice].T @ rhs_k + b)."""
        outs = []
        for m in range(2):
            pm = ps.tile([128, BL], F32, tag="small")
            msl = slice(m * 128, (m + 1) * 128)
            steps = [(wp[:, wsec + k, msl], rhs_list[k]) for k in range(nk)]
            steps.append((bp[0:1, bias_row, msl], ones8[:]))
            mm_chain(pm[:], steps)
            o = sb.tile([128, BL], F32, tag=f"{out_tag}{m}")
            nc.scalar.activation(o[:], pm[:], act)
            outs.append(o)
        return outs

    pctxT = [s3[:, k, :] for k in range(4)]
    hsT = [s3[:, 4 + k, :] for k in range(4)]
    csT = [s3[:, 8 + k, :] for k in range(4)]

    xT = a_matmul(S_LIN, 5, [embT[:]] + pctxT, 0, AF.Copy, "xT")
    hT = a_matmul(S_RH, 4, hsT, 1, AF.Relu, "hT")
    cT = a_matmul(S_RC, 4, csT, 2, AF.Relu, "cT")

    # gates [1024, 8] in 8 chunks of 128; order i,f,g,o
    gact = [AF.Sigmoid, AF.Sigmoid, AF.Sigmoid, AF.Sigmoid,
            AF.Tanh, AF.Tanh, AF.Sigmoid, AF.Sigmoid]
    gate = []
    for m in range(8):
        pm = ps.tile([128, BL], F32, tag="small")
        msl = slice(m * 128, (m + 1) * 128)
        steps = [(gp[:, k, msl], xT[k][:]) for k in range(2)]
        steps += [(gp[:, 2 + k, msl], hT[k][:]) for k in range(2)]
        steps.append((bg[:, msl], ones8[:]))
        mm_chain(pm[:], steps)
        o = sb.tile([128, BL], F32, tag=f"gate{m}")
        nc.scalar.activation(o[:], pm[:], gact[m])
        gate.append(o)

    cnT, hnT = [], []
    for d in range(2):
        i_s, f_s, g_t, o_s = gate[d], gate[2 + d], gate[4 + d], gate[6 + d]
        tmp1 = sbw.tile([128, BL], F32, tag="lstm_t1")
        tmp2 = sbw.tile([128, BL], F32, tag="lstm_t2")
        nc.vector.tensor_mul(tmp1[:], f_s[:], cT[d][:])
        nc.vector.tensor_mul(tmp2[:], i_s[:], g_t[:])
        cn = sb.tile([128, BL], F32, tag=f"cnT{d}")
        nc.vector.tensor_add(cn[:], tmp1[:], tmp2[:])
        tc_t = sbw.tile([128, BL], F32, tag="lstm_tc")
        nc.scalar.activation(tc_t[:], cn[:], AF.Tanh)
        hn = sb.tile([128, BL], F32, tag=f"hnT{d}")
        nc.vector.tensor_mul(hn[:], o_s[:], tc_t[:])
        cnT.append(cn)
        hnT.append(hn)
        nc.sync.dma_start(out=t["h_outT"][d * 128:(d + 1) * 128, :], in_=hn[:])
        nc.sync.dma_start(out=t["c_outT"][d * 128:(d + 1) * 128, :], in_=cn[:])

    # Q/scale
    QsT = []
    for m in range(2):
        pm = ps.tile([128, BL], F32, tag="small")
        msl = slice(m * 128, (m + 1) * 128)
        steps = [(wp[:, S_WQ + k, msl], hnT[k][:]) for k in range(2)]
        steps += [(wp[:, S_WQ + 2 + k, msl], cnT[k][:]) for k in range(2)]
        steps.append((bp[0:1, 3, msl], ones8[:]))
        mm_chain(pm[:], steps)
        q = sb.tile([128, BL], F32, tag=f"QsT{m}")
        nc.scalar.activation(q[:], pm[:], AF.Tanh)
        nc.vector.tensor_scalar_mul(q[:], q[:], 1.0 / float(np.sqrt(D)))
        QsT.append(q)

    # K projection -> K_T [128, 3200] x2 (tanh applied)
    K_T = [pa.tile([128, BLL], F32, tag=f"K_T{m}", name=f"K_T{m}")
           for m in range(2)]
    off = 0
    for nw in KP_NW:
        nsl = slice(off, off + nw)
        for m in range(2):
            msl = slice(m * 128, (m + 1) * 128)
            pm = ps_k.tile([128, 512], F32, tag="kp_ps")
            steps = [(wp[:, S_WK + k, msl], efm[:, k, nsl]) for k in range(4)]
            steps.append((uvw[:, msl], efm[0:2, 4, nsl]))
            mm_chain(pm[:, :nw], steps)
            nc.scalar.activation(K_T[m][:, nsl], pm[:, :nw], AF.Tanh)
        off += nw

    # scores + softmax.  Engine writes must start at partition 0, so per-row
    # results live in a flat [1, 8*512] tile; one DMA reshapes to [8, 512].
    exp_flat = sb.tile([1, BL * 512], F32)
    nc.vector.memset(exp_flat[:], 0.0)
    nmx_f = sbw.tile([1, BL], F32)
    ssum_f = sbw.tile([1, BL], F32)
    for b in range(BL):
        bsl = slice(b * L, (b + 1) * L)
        sc_ps = ps.tile([1, L], F32, tag="small")
        for k in range(2):
            nc.tensor.matmul(sc_ps[:], QsT[k][:, b:b + 1],
                             K_T[k][:, bsl], start=(k == 0), stop=(k == 1))
        nc.vector.tensor_reduce(nmx_f[:, b:b + 1], sc_ps[:],
                                axis=mybir.AxisListType.X,
                                op=mybir.AluOpType.max, negate=True)
        nc.scalar.activation(exp_flat[:, b * 512:b * 512 + L], sc_ps[:],
                             AF.Exp, bias=nmx_f[:, b:b + 1],
                             accum_out=ssum_f[:, b:b + 1])
    att = sb.tile([BL, 512], F32)
    nc.sync.dma_start(out=att[:], in_=exp_flat[:])
    sum_ps = ps.tile([BL, 1], F32, tag="small")
    nc.tensor.transpose(sum_ps[:], ssum_f[:], ident[:1, :1])
    rsum = sbw.tile([BL, 1], F32)
    nc.vector.reciprocal(rsum[:], sum_ps[:])
    nc.vector.tensor_scalar_mul(att[:, :L], att[:, :L], rsum[:])
    nc.sync.dma_start(out=t["att_out"][:], in_=att[:, :L])

    # att transposed chunks [128, 8] x4
    attT = []
    for k in range(4):
        pt = tr(att[:, k * 128:(k + 1) * 128], 128, BL)
        a = sb.tile([128, BL], F32, tag=f"attT{k}")
        nc.scalar.copy(out=a[:], in_=pt[:])
        attT.append(a)

    # weighted context: per-row into flat [1, 8*512], then reshape to [8, 512]
    wc_flat = sb.tile([1, BL * 512], F32)
    for b in range(BL):
        erm = est.tile([128, 4, 512], F32, tag="erm")
        nc.scalar.dma_start(
            out=erm[:],
            in_=t["enc_rmp"][b * 512:(b + 1) * 512, :].rearrange(
                "(k p) f -> p k f", p=128))
        pw = ps.tile([1, 512], F32, tag="small")
        steps = [(attT[k][:, b:b + 1], erm[:, k, :]) for k in range(4)]
        mm_chain(pw[:], steps)
        nc.scalar.copy(out=wc_flat[:, b * 512:(b + 1) * 512], in_=pw[:])
    wc_rm = sb.tile([BL, 2 * D], F32)
    nc.sync.dma_start(out=wc_rm[:], in_=wc_flat[:])
    nc.sync.dma_start(out=t["wc_out"][:], in_=wc_rm[:])

    wcT = []
    for k in range(4):
        pt = tr(wc_rm[:, k * 128:(k + 1) * 128], 128, BL)
        w = sb.tile([128, BL], F32, tag=f"wcT{k}")
        nc.scalar.copy(out=w[:], in_=pt[:])
        wcT.append(w)

    # p_gen [8, 1]
    pp = ps.tile([BL, 1], F32, tag="small")
    comps = [xT[0], xT[1], wcT[0], wcT[1], wcT[2], wcT[3],
             hnT[0], hnT[1], cnT[0], cnT[1]]
    steps = [(comps[k][:], pgw[:, k:k + 1]) for k in range(10)]
    mm_chain(pp[:], steps)
    pgen = sb.tile([BL, 1], F32)
    nc.scalar.activation(pgen[:], pp[:], AF.Sigmoid, bias=pgb[:, 0:1])
    ompgen = sbw.tile([BL, 1], F32)
    nc.scalar.activation(ompgen[:], pp[:], AF.Sigmoid, scale=-1.0,
                         bias=pgb[:, 1:2])
    nc.sync.dma_start(out=t["pgen_out"][:], in_=pgen[:])

    # pregen [256, 8] feature-major
    pregenT = []
    for m in range(2):
        pm = ps.tile([128, BL], F32, tag="small")
        msl = slice(m * 128, (m + 1) * 128)
        steps = [(wp[:, S_PRE + k, msl], wcT[k][:]) for k in range(4)]
        steps += [(wp[:, S_PRE + 4 + k, msl], hnT[k][:]) for k in range(2)]
        steps.append((bp[0:1, 4, msl], ones8[:]))
        mm_chain(pm[:], steps)
        pg_ = sb.tile([128, BL], F32, tag=f"pregenT{m}")
        nc.scalar.copy(out=pg_[:], in_=pm[:])
        pregenT.append(pg_)

    # phase-A heavyweights are dead past this point; release their SBUF
    est_cm.__exit__(None, None, None)
    pa_cm.__exit__(None, None, None)
    gst_cm = tc.tile_pool(name="gst", bufs=4)
    gst = gst_cm.__enter__()
    ctxs.append(gst_cm)
    pb_cm = tc.tile_pool(name="pb", bufs=1)
    pb = pb_cm.__enter__()
    ctxs.append(pb_cm)

    # bundle [8, 657] = [att_dist(400) | pregen(256) | pgen(1)]
    bundle = sb.tile([BL, 657], F32)
    nc.vector.tensor_scalar_mul(bundle[:, :L], att[:, :L], ompgen[:])
    for m in range(2):
        pt = tr(pregenT[m][:], BL, 128)
        nc.scalar.copy(out=bundle[:, L + m * 128:L + (m + 1) * 128], in_=pt[:])
    nc.vector.tensor_copy(bundle[:, 656:657], pgen[:])

    bb_in = dram.tile([BL, 657], F32)
    bb_out = dram.tile([B, 657], F32)
    nc.sync.dma_start(out=bb_in[:], in_=bundle[:])
    nc.gpsimd.collective_compute(
        "AllGather", mybir.AluOpType.bypass,
        replica_groups=[list(range(NCORES))],
        ins=[bb_in.opt()], outs=[bb_out.opt()])
    ball = sb.tile([B, 657], F32)
    nc.sync.dma_start(out=ball[:], in_=bb_out[:])

    att_all = ball[:, :L]
    pgen_all = ball[:, 656:657]
    pT = []
    for m in range(2):
        pt = ps.tile([128, B], F32, tag="small")
        nc.tensor.transpose(pt[:], ball[:, L + m * 128:L + (m + 1) * 128],
                            ident[:B, :B])
        p_ = sb.tile([128, B], F32, tag=f"pT{m}")
        nc.scalar.copy(out=p_[:], in_=pt[:])
        pT.append(p_)

    # ---------------- phase B (transposed: finT [VPAD, 64]) ----------------
    ones128 = pb.tile([128, 1], F32)
    nc.vector.memset(ones128[:], 1.0)
    ones1r = pb.tile([1, 128], F32)
    nc.vector.memset(ones1r[:], 1.0)
    gb = pb.tile([1, VPAD], F32)
    nc.sync.dma_start(out=gb[:], in_=t["gen_aug"][256:257, :])

    expT = pb.tile([128, NMCH, B], F32)          # exp(logits).T chunks
    psum_sum = ps_sum.tile([1, B], F32, tag="sum")
    for m in range(NMCH):
        mw = slice(m * 128, (m + 1) * 128)
        g2 = gst.tile([128, 2, 128], F32, tag="g2")
        nc.scalar.dma_start(out=g2[:], in_=t["gen_aug"][0:256, mw].rearrange(
            "(k p) f -> p k f", p=128))
        pv = ps_v.tile([128, B], F32, tag="gen_ps")
        nc.tensor.matmul(pv[:], g2[:, 0, :], pT[0][:], start=True, stop=False)
        nc.tensor.matmul(pv[:], g2[:, 1, :], pT[1][:], start=False, stop=False)
        nc.tensor.matmul(pv[:], gb[:, mw], ones64[:], start=False, stop=True)
        nc.scalar.activation(expT[:, m, :], pv[:], AF.Exp)
        nc.tensor.matmul(psum_sum[:], ones128[:], expT[:, m, :],
                         start=(m == 0), stop=(m == NMCH - 1))

    lsum = sbw.tile([1, B], F32)
    nc.scalar.copy(out=lsum[:], in_=psum_sum[:])
    ar_in = dram.tile([1, B], F32)
    ar_out = dram.tile([1, B], F32)
    nc.sync.dma_start(out=ar_in[:], in_=lsum[:])
    nc.gpsimd.collective_compute(
        "AllReduce", mybir.AluOpType.add,
        replica_groups=[list(range(NCORES))],
        ins=[ar_in.opt()], outs=[ar_out.opt()])
    gsum = sbw.tile([1, B], F32)
    nc.sync.dma_start(out=gsum[:], in_=ar_out[:])

    # scale_rep[p, b] = p_gen[b] / gsum[b], replicated over partitions
    rg = sbw.tile([1, B], F32)
    nc.vector.reciprocal(rg[:], gsum[:])
    pgt_ps = ps.tile([1, B], F32, tag="small")
    nc.tensor.transpose(pgt_ps[:], pgen_all, ident[:B, :B])
    srow = sbw.tile([1, B], F32)
    nc.vector.tensor_mul(srow[:], pgt_ps[:], rg[:])
    srep_ps = ps.tile([128, B], F32, tag="small")
    nc.tensor.matmul(srep_ps[:], ones1r[:], srow[:], start=True, stop=True)
    scale_rep = pb.tile([128, B], F32)
    nc.vector.tensor_copy(scale_rep[:], srep_ps[:])
    for m in range(NMCH):
        nc.vector.tensor_mul(expT[:, m, :], expT[:, m, :], scale_rep[:])

    # attention scatter rows: V[j, b] = att[b, l_j] * (b == b_j), via
    # one-hot matmul against att.T, then row-scatter-add into finT.
    AT_W = [128, 128, 128, 16]
    attT_bf = []
    for k in range(4):
        a = pb.tile([128, B], mybir.dt.bfloat16, tag=f"attTb{k}")
        if AT_W[k] < 128:
            nc.vector.memset(a[:], 0.0)
        pt = ps.tile([AT_W[k], B], F32, tag="small")
        nc.tensor.transpose(pt[:], att_all[:, k * 128:k * 128 + AT_W[k]],
                            ident[:B, :B])
        nc.scalar.copy(out=a[0:AT_W[k], :], in_=pt[:])
        attT_bf.append(a)

    io64 = pb.tile([128, B], F32)
    nc.sync.dma_start(out=io64[:], in_=t["iota64"][:])
    ntiles = sum(layer_tiles)
    bsel_sb = pb.tile([128, ntiles], F32)
    nc.sync.dma_start(out=bsel_sb[:], in_=t["bsel"][:].rearrange(
        "(t p) o -> p (t o)", p=128))
    ridx_sb = pb.tile([128, ntiles], I32)
    nc.sync.dma_start(out=ridx_sb[:], in_=t["rowidx"][:].rearrange(
        "(t p) o -> p (t o)", p=128))

    V_sb = pb.tile([128, ntiles, B], F32)
    scatters = []
    prev_layer = []
    ti = 0
    for lt in layer_tiles:
        cur = []
        for _ in range(lt):
            sp = gst.tile([128, 4, 128], mybir.dt.bfloat16, tag="spk")
            nc.scalar.dma_start(
                out=sp[:],
                in_=t["s_pack"][ti * 512:(ti + 1) * 512, :].rearrange(
                    "(k p) m -> p k m", p=128))
            gp_ = ps.tile([128, B], F32, tag="small")
            for k in range(4):
                nc.tensor.matmul(gp_[:], sp[:, k, :], attT_bf[k][:],
                                 start=(k == 0), stop=(k == 3))
            nc.vector.scalar_tensor_tensor(
                V_sb[:, ti, :], io64[:], bsel_sb[:, ti:ti + 1], gp_[:],
                op0=mybir.AluOpType.is_equal, op1=mybir.AluOpType.mult)
            sc = nc.gpsimd.indirect_dma_start(
                out=t["finT"][:],
                out_offset=bass.IndirectOffsetOnAxis(
                    ap=ridx_sb[:, ti:ti + 1], axis=0),
                in_=V_sb[:, ti, :], in_offset=None,
                bounds_check=VPAD - 1, oob_is_err=False,
                compute_op=mybir.AluOpType.add)
            for pl in prev_layer:
                add_dep_helper(sc.ins, pl.ins, reason="scatter layer order")
            cur.append(sc)
            scatters.append(sc)
            ti += 1
        prev_layer = cur

    # dense part: finT += scale * expT (after all scatter-adds)
    wr = nc.gpsimd.dma_start(
        out=t["finT"][:].rearrange("(m p) b -> p m b", p=128),
        in_=expT[:], accum_op=mybir.AluOpType.add)
    for sc in scatters:
        add_dep_helper(wr.ins, sc.ins, reason="dense add after scatters")

    for p in reversed(ctxs):
        p.__exit__(None, None, None)


# --------------------------------------------------------------------------
# host-side: input prep, launch, unshard
# --------------------------------------------------------------------------

def _prep_shared(w):
    """Preprocess weights (shared across cores)."""
    f = np.float32
    lin_wT = np.ascontiguousarray(w["lin_w"].T, f)           # [640, 256]
    redh = np.ascontiguousarray(w["red_h_w"].T, f)           # [512, 256]
    redc = np.ascontiguousarray(w["red_c_w"].T, f)
    wq = np.ascontiguousarray(w["wq_w"].T, f)
    wk = np.ascontiguousarray(w["wk_w"].T, f)
    pre = np.ascontiguousarray(w["pre_w"].T, f)              # [768, 256]
    wpack = np.concatenate([lin_wT, redh, redc, wq, wk, pre], 0)
    assert wpack.shape == (27 * 128, 256)

    bias_pack = np.stack([w["lin_b"], w["red_h_b"], w["red_c_b"],
                          w["wq_b"], w["pre_b"]]).astype(f)
    u = (w["wk_w"] @ w["cov_w"][:, 0]).astype(f)
    v = (w["wk_w"] @ w["cov_b"] + w["wk_b"]).astype(f)
    uv_wk = np.stack([u, v])

    gpack = np.concatenate([np.ascontiguousarray(w["w_ih"].T, f),
                            np.ascontiguousarray(w["w_hh"].T, f)], 0)
    bias_g = (w["b_ih"] + w["b_hh"]).astype(f)[None, :]
    pgen_wT = np.ascontiguousarray(w["pgen_w"].T, f)         # [1280, 1]
    pb = float(w["pgen_b"][0])
    pgen_bb = np.tile(np.array([[pb, -pb]], f), (BL, 1))

    emb_w = np.ascontiguousarray(w["emb_w"], f)
    gen_w = np.ascontiguousarray(w["gen_w"], f)
    gen_b = np.asarray(w["gen_b"], f)
    gen_augs = []
    for c in range(NCORES):
        lo = c * VSL
        ga = np.zeros((257, VPAD), f)
        ga[256, :] = NEG
        ncols = min(VSL, max(0, V - lo))
        if ncols > 0:
            ga[:256, :ncols] = gen_w[lo:lo + ncols, :].T
            ga[256, :ncols] = gen_b[lo:lo + ncols]
        gen_augs.append(ga)
    return dict(wpack=wpack, bias_pack=bias_pack, uv_wk=uv_wk, gpack=gpack,
                bias_g=bias_g, pgen_wT=pgen_wT, pgen_bb=pgen_bb,
                emb_w=emb_w, gen_augs=gen_augs)


def _prep_scatter(extended_input):
    """Layered scatter-tile plan per core.

    Items (b, l) grouped by target vocab column; occurrence p of a column goes
    to layer p (layers' scatter DMAs are serialized so add-RMW never races).
    Returns per-core dict(s_pack, bsel, rowidx) + the shared layer_tiles key.
    """
    ei = np.asarray(extended_input).astype(np.int64)
    core_of = ei // VSL
    local = (ei - core_of * VSL).astype(np.int64)

    # per core: layers -> list of (col, b, l)
    layers = {c: [] for c in range(NCORES)}
    seen = {c: {} for c in range(NCORES)}
    for b in range(B):
        for l in range(L):
            c = int(core_of[b, l])
            col = int(local[b, l])
            p = seen[c].get(col, 0)
            seen[c][col] = p + 1
            while len(layers[c]) <= p:
                layers[c].append([])
            layers[c][p].append((col, b, l))

    nlayers = max(len(layers[c]) for c in range(NCORES))
    layer_tiles = []
    for p in range(nlayers):
        mx = max(len(layers[c][p]) if len(layers[c]) > p else 0
                 for c in range(NCORES))
        layer_tiles.append(max(1, -(-mx // 128)))
    layer_tiles = tuple(layer_tiles)
    ntiles = sum(layer_tiles)

    out = {}
    for c in range(NCORES):
        s_pack = np.zeros((ntiles * 512, 128), np.float32)
        bsel = np.zeros((ntiles * 128, 1), np.float32)
        rowidx = np.full((ntiles * 128, 1), OOB, np.int32)
        t0 = 0
        for p, lt in enumerate(layer_tiles):
            items = layers[c][p] if len(layers[c]) > p else []
            for j, (col, b, l) in enumerate(items):
                tt = t0 + j // 128
                sl = j % 128
                rowidx[tt * 128 + sl, 0] = col
                bsel[tt * 128 + sl, 0] = b
                s_pack[tt * 512 + l, sl] = 1.0
            t0 += lt
        import ml_dtypes
        out[c] = dict(s_pack=s_pack.astype(ml_dtypes.bfloat16),
                      bsel=bsel, rowidx=rowidx)
    return out, layer_tiles


def kernel(**inputs):
    f = np.float32
    inp = {k: np.asarray(v) for k, v in inputs.items()}
    shared = _prep_shared(inp)
    scat, layer_tiles = _prep_scatter(inp["extended_input"])
    iota64 = np.tile(np.arange(B, dtype=f)[None, :], (128, 1))

    enc = np.ascontiguousarray(inp["encoder_output"], f)     # [64, 400, 512]
    pctx = np.asarray(inp["previous_context"], f)[:, 0, :]   # [64, 512]
    hh = np.asarray(inp["hidden_h"], f)                      # [2, 64, 256]
    hc = np.asarray(inp["hidden_c"], f)
    covr = np.asarray(inp["coverage"], f)                    # [64, 400]
    ids_full = np.asarray(inp["input"]).astype(np.int32)     # [64, 1]

    in_maps = []
    for c in range(NCORES):
        rows = slice(c * BL, (c + 1) * BL)
        e = enc[rows]                                        # [8, 400, 512]
        enc_fm = np.ascontiguousarray(
            e.reshape(BLL, 512).T)                           # [512, 3200]
        enc_rmp = np.zeros((BL * 512, 512), f)
        for b in range(BL):
            enc_rmp[b * 512:b * 512 + L] = e[b]
        cov_aug = np.ones((2, BLL), f)
        cov_aug[0] = covr[rows].reshape(-1)
        small3 = np.concatenate([
            np.ascontiguousarray(pctx[rows].T),              # [512, 8]
            np.concatenate([hh[0, rows].T, hh[1, rows].T], 0),
            np.concatenate([hc[0, rows].T, hc[1, rows].T], 0)], 0)
        in_maps.append(dict(
            ids=ids_full[rows], small3=small3, cov_aug=cov_aug,
            enc_fm=enc_fm, enc_rmp=enc_rmp, emb_w=shared["emb_w"],
            wpack=shared["wpack"], bias_pack=shared["bias_pack"],
            uv_wk=shared["uv_wk"], gpack=shared["gpack"],
            bias_g=shared["bias_g"], pgen_wT=shared["pgen_wT"],
            pgen_bb=shared["pgen_bb"], gen_aug=shared["gen_augs"][c],
            iota64=iota64, **scat[c]))

    if layer_tiles not in _CACHE:
        _CACHE[layer_tiles] = _build(layer_tiles)
    nc = _CACHE[layer_tiles]
    global LAST
    LAST = run_bass_kernel_spmd(nc, in_maps, list(range(NCORES)), trace=TRACE)
    res = LAST.results

    final = np.concatenate(
        [res[c]["finT"][:VSL, :].T for c in range(NCORES)], 1)
    h = np.concatenate([res[c]["h_outT"].T for c in range(NCORES)], 0)[None]
    cc = np.concatenate([res[c]["c_outT"].T for c in range(NCORES)], 0)[None]
    wc = np.concatenate([res[c]["wc_out"] for c in range(NCORES)], 0)[:, None, :]
    att = np.concatenate([res[c]["att_out"] for c in range(NCORES)], 0)
    pgen = np.concatenate([res[c]["pgen_out"] for c in range(NCORES)], 0)
    coverage = np.asarray(inp["coverage"], f)
    return (final, (h, cc), wc, att, coverage, pgen)
